# revision 13
# baseline (speedup 1.0000x reference)
# Trainium2 Bass kernel for nn_ColorConsistencyMetric.
#
# Reference computation (B=32, C=3, H=W=1024, GRID=4):
#   region_means[b,c,gi,gj] = mean of the 256x256 block (gi,gj) of images[b,c]
#   color_std[b] = mean_c std(region_means[b,c,:], ddof=1)
#   out = mean_b 1/(1+color_std[b])
#
# Strategy: pure data parallel over the batch dim across 8 NeuronCores
# (4 images per core). Each core streams its 48 MiB shard through SBUF
# (12 channel-images, one 4 MiB DMA each as a [128, 8192] tile: partition
# p holds image rows 8p..8p+7), computes per-(row-in-partition, col-block)
# sums with a single segmented VectorE reduce, reduces across partitions
# with a block-diagonal ones matmul on TensorE (partitions 32m..32m+31 all
# belong to block-row m), and a tiny second reduce yields the 16 block
# sums per channel-image. The 8x[4,48] outputs (one f32 per block per
# channel-image) are combined on the host: block mean -> std -> 1/(1+std)
# -> mean over batch. The kernel is HBM-bandwidth bound (~48 MiB/core).

import numpy as np

_B, _C, _H, _W = 32, 3, 1024, 1024
_GRID = 4
_NCORES = 8
_BPC = _B // _NCORES            # images per core
_NIMG = _BPC * _C               # channel-images per core
_RPP = _H // 128                # image rows per SBUF partition
_FD = _RPP * _W                 # free dim of one channel-image tile
_BLK = (_H // _GRID) * (_W // _GRID)  # pixels per block

_cache = {}


def _build_bass(repeats=1):
    """repeats>1 re-runs the whole per-core workload inside one program;
    used by test.py to difference out the host->device dispatch overhead
    when timing. kernel() always uses repeats=1."""
    import concourse.bass as bass
    import concourse.bacc as bacc
    import concourse.tile as tile
    from concourse import mybir

    nc = bacc.Bacc(
        "TRN2",
        target_bir_lowering=False,
        debug=False,
        num_devices=_NCORES,
    )
    imgs = nc.dram_tensor(
        "images", [_NIMG, 128, _FD], mybir.dt.float32, kind="ExternalInput"
    ).ap()
    out = nc.dram_tensor(
        "blocksums",
        [_GRID, _NIMG * _GRID * repeats],
        mybir.dt.float32,
        kind="ExternalOutput",
    ).ap()

    with tile.TileContext(nc) as tc:
        with (
            tc.tile_pool(name="big", bufs=4) as big,
            tc.tile_pool(name="psum", bufs=2, space="PSUM") as psum_pool,
            tc.tile_pool(name="const", bufs=1) as const_pool,
            tc.tile_pool(name="outp", bufs=1) as outp,
        ):
            # Block-diagonal ones: lhsT[p, m] = 1 iff p // 32 == m, so the
            # matmul sums partitions within each block-row group (all 8
            # image rows held by a partition are in the same block-row).
            lhsT = const_pool.tile([128, _GRID], mybir.dt.float32)
            nc.vector.memset(lhsT, 0.0)
            for m in range(_GRID):
                nc.vector.memset(lhsT[m * 32 : (m + 1) * 32, m : m + 1], 1.0)

            W = _NIMG * _GRID
            rs = outp.tile([128, W * repeats], mybir.dt.float32)

            for k in range(_NIMG * repeats):
                i = k % _NIMG
                t = big.tile([128, _FD], mybir.dt.float32)
                nc.sync.dma_start(out=t, in_=imgs[i])
                # Sum rows-in-partition and cols within each col-block:
                # rs[p, k*4+j] = sum of image i's col-block j in partition p.
                nc.vector.reduce_sum(
                    out=rs[:, k * _GRID : (k + 1) * _GRID],
                    in_=t.rearrange("p (r j c) -> p j r c", r=_RPP, j=_GRID),
                    axis=mybir.AxisListType.XY,
                )
            for r in range(repeats):
                # Sum the 128 partitions within each block-row group.
                ps = psum_pool.tile([_GRID, W], mybir.dt.float32)
                nc.tensor.matmul(
                    ps, lhsT, rs[:, r * W : (r + 1) * W], start=True, stop=True
                )
                osb = outp.tile([_GRID, W], mybir.dt.float32)
                nc.vector.tensor_copy(osb, ps)
                nc.sync.dma_start(
                    out=out[:, r * W : (r + 1) * W], in_=osb
                )
    nc.compile()
    return nc


def _get_nc(repeats=1):
    key = ("nc", repeats)
    if key not in _cache:
        _cache[key] = _build_bass(repeats)
    return _cache[key]


def _run_on_device(images_np, trace=False, **spmd_kwargs):
    from concourse.bass_utils import run_bass_kernel_spmd

    nc = _get_nc()
    in_maps = []
    for c in range(_NCORES):
        shard = np.ascontiguousarray(
            images_np[c * _BPC : (c + 1) * _BPC], dtype=np.float32
        ).reshape(_NIMG, 128, _FD)
        in_maps.append({"images": shard})
    res = run_bass_kernel_spmd(
        nc, in_maps, core_ids=list(range(_NCORES)), trace=trace, **spmd_kwargs
    )
    return res


def _finish_host(block_sum_list):
    """block_sum_list: per-core [GRID, NIMG*GRID] arrays of block sums."""
    cons = []
    for o in block_sum_list:
        # o[gi, i*GRID + gj] with i = local_b * C + c
        M = np.asarray(o, dtype=np.float64).reshape(_GRID, _NIMG, _GRID)
        sums = M.transpose(1, 0, 2)                      # (i, gi, gj)
        means = (sums / _BLK).reshape(_BPC, _C, _GRID * _GRID)
        mu = means.mean(axis=-1, keepdims=True)
        var = ((means - mu) ** 2).sum(axis=-1) / (_GRID * _GRID - 1)
        std = np.sqrt(var)                               # (b, c)
        color_std = std.mean(axis=1)                     # (b,)
        cons.append(1.0 / (1.0 + color_std))
    return np.array(np.concatenate(cons).mean(), dtype=np.float32)


def kernel(images):
    images_np = np.asarray(images)
    res = _run_on_device(images_np, trace=False)
    outs = [r["blocksums"] for r in res.results]
    return _finish_host(outs)


# revision 33
# speedup vs baseline: 1.0929x; 1.0929x over previous
# Trainium2 Bass kernel for nn_ColorConsistencyMetric.
#
# Reference computation (B=32, C=3, H=W=1024, GRID=4):
#   region_means[b,c,gi,gj] = mean of the 256x256 block (gi,gj) of images[b,c]
#   color_std[b] = mean_c std(region_means[b,c,:], ddof=1)
#   out = mean_b 1/(1+color_std[b])
#
# Strategy: pure data parallel over the batch dim across 8 NeuronCores
# (4 images per core). Each core streams its 48 MiB shard through SBUF
# (12 channel-images, one 4 MiB DMA each as a [128, 8192] tile: partition
# p holds image rows 8p..8p+7), computes per-(row-in-partition, col-block)
# sums with a single segmented VectorE reduce, reduces across partitions
# with a block-diagonal ones matmul on TensorE (partitions 32m..32m+31 all
# belong to block-row m), and a tiny second reduce yields the 16 block
# sums per channel-image. The 8x[4,48] outputs (one f32 per block per
# channel-image) are combined on the host: block mean -> std -> 1/(1+std)
# -> mean over batch. The kernel is HBM-bandwidth bound (~48 MiB/core).

import numpy as np

_B, _C, _H, _W = 32, 3, 1024, 1024
_GRID = 4
_NCORES = 8
_BPC = _B // _NCORES            # images per core
_NIMG = _BPC * _C               # channel-images per core
_RPP = _H // 128                # image rows per SBUF partition
_FD = _RPP * _W                 # free dim of one channel-image tile
_BLK = (_H // _GRID) * (_W // _GRID)  # pixels per block

_cache = {}
_PROD_MODE = "base"  # mode kernel() uses; see _build_bass


def _build_bass(repeats=1, mode="base"):
    """repeats>1 re-runs the whole per-core workload inside one program;
    used by test.py to difference out the host->device dispatch overhead
    when timing. kernel() always uses repeats=1.
    mode: "base"  - 12x 4MiB loads on nc.sync, bufs=4
          "dual"  - loads alternate nc.sync / nc.scalar HWDGE rings
          "bufs5" - like base with 5 slot buffers
          "bufs6" - like base with 6 slot buffers"""
    import concourse.bass as bass
    import concourse.bacc as bacc
    import concourse.tile as tile
    from concourse import mybir

    nc = bacc.Bacc(
        "TRN2",
        target_bir_lowering=False,
        debug=False,
        num_devices=_NCORES,
    )
    imgs = nc.dram_tensor(
        "images", [_NIMG, 128, _FD], mybir.dt.float32, kind="ExternalInput"
    ).ap()
    out_shape = (
        [2 * _GRID, (_NIMG // 2) * _GRID * repeats]
        if mode == "big2"
        else [_GRID, _NIMG * _GRID * repeats]
    )
    out = nc.dram_tensor(
        "blocksums", out_shape, mybir.dt.float32, kind="ExternalOutput"
    ).ap()

    from contextlib import ExitStack

    if mode == "big2":
        return _build_bass_big2(nc, bass, tile, mybir, imgs, out, repeats)
    nbufs = {
        "base": 4, "dual": 4, "bufs5": 5, "bufs6": 6, "acttail": 4,
        "tri": 4, "dualg": 4, "dual5": 5, "dualat": 4, "tsall": 4,
    }[mode]
    # Images whose column sums ScalarE computes (activation accum_out)
    # instead of VectorE, so the tail after the last DMA is shorter and
    # DVE sheds work. ACT does 4 ops per image (one per col-block).
    act_imgs = {9, 10, 11} if mode in ("acttail", "dualat") else set()
    with tile.TileContext(nc) as tc:
        with ExitStack() as ctx:
            big = ctx.enter_context(tc.tile_pool(name="big", bufs=nbufs))
            psum_pool = ctx.enter_context(
                tc.tile_pool(name="psum", bufs=2, space="PSUM")
            )
            const_pool = ctx.enter_context(tc.tile_pool(name="const", bufs=1))
            outp = ctx.enter_context(tc.tile_pool(name="outp", bufs=1))
            dummyp = (
                ctx.enter_context(tc.tile_pool(name="dummy", bufs=2))
                if (act_imgs or mode == "tsall")
                else None
            )
            # Block-diagonal ones: lhsT[p, m] = 1 iff p // 32 == m, so the
            # matmul sums partitions within each block-row group (all 8
            # image rows held by a partition are in the same block-row).
            lhsT = const_pool.tile([128, _GRID], mybir.dt.float32)
            nc.vector.memset(lhsT, 0.0)
            for m in range(_GRID):
                nc.vector.memset(lhsT[m * 32 : (m + 1) * 32, m : m + 1], 1.0)

            W = _NIMG * _GRID
            rs = outp.tile([128, W * repeats], mybir.dt.float32)

            for k in range(_NIMG * repeats):
                i = k % _NIMG
                t = big.tile([128, _FD], mybir.dt.float32)
                if mode in ("dual", "dual5", "tsall"):
                    eng = nc.scalar if k % 2 else nc.sync
                elif mode == "tri":
                    eng = (nc.sync, nc.scalar, nc.gpsimd)[k % 3]
                elif mode in ("dualg", "dualat"):
                    eng = nc.gpsimd if k % 2 else nc.sync
                else:
                    eng = nc.sync
                eng.dma_start(out=t, in_=imgs[i])
                # Sum rows-in-partition and cols within each col-block:
                # rs[p, k*4+j] = sum of image i's col-block j in partition p.
                tv = t.rearrange("p (r j c) -> p j r c", r=_RPP, j=_GRID)
                if mode == "tsall":
                    dummy = (dummyp or big).tile(
                        [128, _RPP * 256], mybir.dt.float32, tag="dummy"
                    )
                    for j in range(_GRID):
                        nc.vector.tensor_scalar(
                            out=dummy,
                            in0=tv[:, j],
                            scalar1=1.0,
                            scalar2=None,
                            op0=mybir.AluOpType.mult,
                            accum_out=rs[
                                :, k * _GRID + j : k * _GRID + j + 1
                            ],
                        )
                elif i in act_imgs:
                    dummy = dummyp.tile([128, _RPP * 256], mybir.dt.float32)
                    for j in range(_GRID):
                        nc.scalar.activation(
                            out=dummy,
                            in_=tv[:, j],
                            func=mybir.ActivationFunctionType.Copy,
                            accum_out=rs[
                                :, k * _GRID + j : k * _GRID + j + 1
                            ],
                        )
                else:
                    nc.vector.reduce_sum(
                        out=rs[:, k * _GRID : (k + 1) * _GRID],
                        in_=tv,
                        axis=mybir.AxisListType.XY,
                    )
            for r in range(repeats):
                # Sum the 128 partitions within each block-row group.
                ps = psum_pool.tile([_GRID, W], mybir.dt.float32)
                nc.tensor.matmul(
                    ps, lhsT, rs[:, r * W : (r + 1) * W], start=True, stop=True
                )
                osb = outp.tile([_GRID, W], mybir.dt.float32)
                nc.vector.tensor_copy(osb, ps)
                nc.sync.dma_start(
                    out=out[:, r * W : (r + 1) * W], in_=osb
                )
    nc.compile()
    return nc


def _build_bass_big2(nc, bass, tile, mybir, imgs, out, repeats):
    """2 images per DMA (8 MiB transfers). Partition p holds 16 rows of
    image (pair*2 + p//64); within its image, block-row = (p % 64) // 16.
    lhsT has 8 one-hot groups of 16 partitions -> psum rows g = 4*(p//64)
    + block-row. Output layout per pair q: psum[g, q*4 + j]."""
    from contextlib import ExitStack

    NP = _NIMG // 2  # pairs
    imgs2 = imgs.rearrange("(q two) p f -> q (two p f)", two=2).rearrange(
        "q (p f) -> q p f", p=128
    )
    with tile.TileContext(nc) as tc:
        with ExitStack() as ctx:
            big = ctx.enter_context(tc.tile_pool(name="big", bufs=2))
            psum_pool = ctx.enter_context(
                tc.tile_pool(name="psum", bufs=2, space="PSUM")
            )
            const_pool = ctx.enter_context(tc.tile_pool(name="const", bufs=1))
            outp = ctx.enter_context(tc.tile_pool(name="outp", bufs=1))
            lhsT = const_pool.tile([128, 8], mybir.dt.float32)
            nc.vector.memset(lhsT, 0.0)
            for g in range(8):
                nc.vector.memset(lhsT[g * 16 : (g + 1) * 16, g : g + 1], 1.0)

            W = NP * _GRID  # 24 per repeat
            rs = outp.tile([128, W * repeats], mybir.dt.float32)
            for k in range(NP * repeats):
                q = k % NP
                t = big.tile([128, 2 * _FD], mybir.dt.float32)
                eng = nc.scalar if k % 2 else nc.sync
                eng.dma_start(out=t, in_=imgs2[q])
                nc.vector.reduce_sum(
                    out=rs[:, k * _GRID : (k + 1) * _GRID],
                    in_=t.rearrange(
                        "p (r j c) -> p j r c", r=2 * _RPP, j=_GRID
                    ),
                    axis=mybir.AxisListType.XY,
                )
            for r in range(repeats):
                ps = psum_pool.tile([8, W], mybir.dt.float32)
                nc.tensor.matmul(
                    ps, lhsT, rs[:, r * W : (r + 1) * W], start=True, stop=True
                )
                osb = outp.tile([8, W], mybir.dt.float32)
                nc.vector.tensor_copy(osb, ps)
                nc.sync.dma_start(out=out[:, r * W : (r + 1) * W], in_=osb)
    nc.compile()
    return nc


def _get_nc(repeats=1, mode="base"):
    key = ("nc", repeats, mode)
    if key not in _cache:
        _cache[key] = _build_bass(repeats, mode)
    return _cache[key]


def _run_on_device(images_np, trace=False, **spmd_kwargs):
    from concourse.bass_utils import run_bass_kernel_spmd

    nc = _get_nc(1, _PROD_MODE)
    in_maps = []
    for c in range(_NCORES):
        shard = np.ascontiguousarray(
            images_np[c * _BPC : (c + 1) * _BPC], dtype=np.float32
        ).reshape(_NIMG, 128, _FD)
        in_maps.append({"images": shard})
    res = run_bass_kernel_spmd(
        nc, in_maps, core_ids=list(range(_NCORES)), trace=trace, **spmd_kwargs
    )
    return res


def _finish_host(block_sum_list):
    """block_sum_list: per-core [GRID, NIMG*GRID] arrays of block sums."""
    cons = []
    for o in block_sum_list:
        # o[gi, i*GRID + gj] with i = local_b * C + c
        M = np.asarray(o, dtype=np.float64).reshape(_GRID, _NIMG, _GRID)
        sums = M.transpose(1, 0, 2)                      # (i, gi, gj)
        means = (sums / _BLK).reshape(_BPC, _C, _GRID * _GRID)
        mu = means.mean(axis=-1, keepdims=True)
        var = ((means - mu) ** 2).sum(axis=-1) / (_GRID * _GRID - 1)
        std = np.sqrt(var)                               # (b, c)
        color_std = std.mean(axis=1)                     # (b,)
        cons.append(1.0 / (1.0 + color_std))
    return np.array(np.concatenate(cons).mean(), dtype=np.float32)


def kernel(images):
    images_np = np.asarray(images)
    res = _run_on_device(images_np, trace=False)
    outs = [r["blocksums"] for r in res.results]
    return _finish_host(outs)
